# revision 11
# baseline (speedup 1.0000x reference)
"""Trainium2 Bass kernel for a LoRA-augmented relu-gated MLP.

Math (per reference):
    y1 = x @ w_gate + b_gate + (x @ Ag) @ Bg
    y2 = x @ w_up   + b_up   + (x @ Au) @ Bu
    x3 = relu(y1) * y2
    y3 = x3 @ w_down + b_down + (x3 @ Ad) @ Bd

Strategy:
  * Host folds every LoRA pair into its base matrix (W_eff = W + A@B in
    float64, rounded to f32) so the device kernel is a plain gated MLP.
  * Data parallel over the 8 NeuronCores: 8192 tokens -> 1024 per core,
    every core holds the full (folded) weights.
  * All matmuls run as float32r (full-rate fp32 on the PE when the moving
    dim is >= 256).
  * Per core: x is transposed on-chip (PE transpose) to xT[k, m]; the MLP
    is computed in f-quarters: gate/up produce x3T stripes [128f, NT] in
    SBUF; the down projection consumes them as stationary operands and
    accumulates partial y3 straight into DRAM via accumulate-DMA.
    b_down enters through a rank-1 ones-matmul in the first quarter.
"""

import sys
import types

import numpy as np

# The trimmed container's `antenv` lacks `axon_hooks`; bass_utils imports it
# unconditionally when tracing is requested (e.g. BASS_TRACE=1). Provide the
# degraded no-hook module so tracing falls back gracefully instead of crashing.
try:
    import antenv.axon_hooks  # noqa: F401
except ImportError:
    _m = types.ModuleType("antenv.axon_hooks")
    _m._hook = None
    _m.set_axon_ntff_profile_hook = lambda h: setattr(_m, "_hook", h)
    _m.get_axon_ntff_profile_hook = lambda: _m._hook
    sys.modules["antenv.axon_hooks"] = _m

import concourse.bacc as bacc
import concourse.bass as bass
import concourse.mybir as mybir
import concourse.tile as tile
from concourse.bass_utils import run_bass_kernel_spmd
P = 128
F32 = mybir.dt.float32
F32R = mybir.dt.float32r
AF = mybir.ActivationFunctionType
ALU = mybir.AluOpType


class Cfg:
    def __init__(self, nt=1024, d=2048, f=8192, fq=4, n_cores=8):
        assert nt % P == 0 and d % P == 0 and f % P == 0
        self.NT = nt          # tokens per core
        self.D = d            # model dim
        self.F = f            # ffn dim
        self.KC = d // P      # contraction chunks for gate/up
        self.NF = f // P      # f-tiles
        self.FQ = fq          # f quarters (x3T resident per quarter)
        assert self.NF % fq == 0
        self.SQ = self.NF // fq
        self.MH = min(512, nt)          # moving-dim chunk for gate/up
        self.NMH = nt // self.MH
        self.DC = min(512, d)           # down-proj d chunk
        self.ND = d // self.DC
        self.NM = nt // P               # token chunks of 128
        self.MG = min(8, self.NM)       # psum group size for down-proj
        self.NMG = self.NM // self.MG
        self.N_CORES = n_cores


def build_bass(cfg: Cfg):
    """Builds the per-core Bass program (same program on all cores)."""
    c = cfg
    nc = bacc.Bacc("TRN2", target_bir_lowering=False, debug=False,
                   num_swdge_queues=4)

    xt = nc.dram_tensor("xt", [P, c.KC, c.NT], F32R, kind="ExternalInput")
    wg = nc.dram_tensor("wg", [c.NF, P, c.KC, P], F32R, kind="ExternalInput")
    wu = nc.dram_tensor("wu", [c.NF, P, c.KC, P], F32R, kind="ExternalInput")
    wd = nc.dram_tensor("wd", [c.NF, c.ND, P, c.DC], F32R, kind="ExternalInput")
    bg = nc.dram_tensor("bg", [P, c.NF], F32, kind="ExternalInput")
    bu = nc.dram_tensor("bu", [P, c.NF], F32, kind="ExternalInput")
    # cst row 0: ones (first 128 entries used), row 1: b_down
    cst = nc.dram_tensor("cst", [2, c.D], F32R, kind="ExternalInput")
    y = nc.dram_tensor("y", [c.NT, c.D], F32, kind="ExternalOutput")

    with tile.TileContext(nc) as tc:
        with (
            tc.tile_pool(name="consts", bufs=1) as consts,
            tc.tile_pool(name="wpool", bufs=4) as wpool,
            tc.tile_pool(name="wdpool", bufs=6) as wdpool,
            tc.tile_pool(name="xTp", bufs=1) as xTp,
            tc.tile_pool(name="x3p", bufs=1) as x3p,
            tc.tile_pool(name="actp", bufs=2) as actp,
            tc.tile_pool(name="outp", bufs=10) as outp,
            tc.tile_pool(name="pall", bufs=1, space="PSUM") as pall,
        ):
            bgt = consts.tile([P, c.NF], F32, name="bgt")
            nc.sync.dma_start(bgt, bg[:, :])
            but = consts.tile([P, c.NF], F32, name="but")
            nc.sync.dma_start(but, bu[:, :])
            ones = consts.tile([1, P], F32R, name="ones")
            nc.sync.dma_start(ones, cst[0:1, 0:P])
            bdr = consts.tile([1, c.D], F32R, name="bdr")
            nc.sync.dma_start(bdr, cst[1:2, :])

            # ---- load pre-transposed x: xT[kk, k_idx, m] ----
            xT = xTp.tile([P, c.KC, c.NT], F32R, name="xT")
            for k in range(c.KC):
                nc.sync.dma_start(xT[:, k, :], xt[:, k, :])

            DTAGS = ["p1", "p1", "p2", "p2", "pd0", "pd1", "pd2", "pd3"]
            for q in range(c.FQ):
                # ---- gate/up projections for this f-quarter ----
                x3 = [
                    x3p.tile([P, c.NT], F32R, tag=f"s{s}", name=f"x3_{q}_{s}")
                    for s in range(c.SQ)
                ]
                for s in range(c.SQ):
                    ft = q * c.SQ + s
                    wgt = wpool.tile([P, c.KC, P], F32R, tag="w", name=f"wg{ft}")
                    nc.sync.dma_start(wgt, wg[ft])
                    wut = wpool.tile([P, c.KC, P], F32R, tag="w", name=f"wu{ft}")
                    nc.sync.dma_start(wut, wu[ft])
                    for h in range(c.NMH):
                        msl = slice(h * c.MH, (h + 1) * c.MH)
                        p1 = pall.tile([P, c.MH], F32, tag="p1", bufs=2,
                                       name=f"p1_{ft}_{h}")
                        p2 = pall.tile([P, c.MH], F32, tag="p2", bufs=2,
                                       name=f"p2_{ft}_{h}")
                        for k in range(c.KC):
                            nc.tensor.matmul(
                                p1, wgt[:, k, :],
                                xT[:, k, msl],
                                start=(k == 0), stop=(k == c.KC - 1))
                        for k in range(c.KC):
                            nc.tensor.matmul(
                                p2, wut[:, k, :],
                                xT[:, k, msl],
                                start=(k == 0), stop=(k == c.KC - 1))
                        t1 = actp.tile([P, c.MH], F32, tag="t1", name=f"t1_{ft}_{h}")
                        nc.scalar.activation(t1, p1, AF.Relu, bias=bgt[:, ft:ft + 1])
                        # x3 = (p2 + b_up) * relu(p1 + b_gate)
                        nc.vector.scalar_tensor_tensor(
                            x3[s][:, msl], p2, but[:, ft:ft + 1], t1,
                            op0=ALU.add, op1=ALU.mult)
                # ---- down projection partials for this f-quarter ----
                for d in range(c.ND):
                    dsl = slice(d * c.DC, (d + 1) * c.DC)
                    for g in range(c.NMG):
                        pds = [
                            pall.tile([P, c.DC], F32, tag=DTAGS[j],
                                      bufs=2 if DTAGS[j] in ("p1", "p2") else 1,
                                      name=f"pd_{q}_{d}_{g}_{j}")
                            for j in range(c.MG)
                        ]
                        if q == 0:
                            # seed psum with b_down via rank-1 ones matmul
                            for j in range(c.MG):
                                nc.tensor.matmul(
                                    pds[j], ones[:, 0:P], bdr[:, dsl],
                                    start=True, stop=False)
                        for s in range(c.SQ):
                            wdt = wdpool.tile([P, c.DC], F32R, tag="wd",
                                              name=f"wd_{q}_{d}_{g}_{s}")
                            nc.sync.dma_start(wdt, wd[q * c.SQ + s, d])
                            for j in range(c.MG):
                                m = g * c.MG + j
                                nc.tensor.matmul(
                                    pds[j],
                                    x3[s][:, m * P:(m + 1) * P],
                                    wdt,
                                    start=(s == 0 and q != 0),
                                    stop=(s == c.SQ - 1))
                        for j in range(c.MG):
                            m = g * c.MG + j
                            ot = outp.tile([P, c.DC], F32, tag="ot",
                                           name=f"ot_{q}_{d}_{g}_{j}")
                            if j % 2 == 0:
                                nc.vector.tensor_copy(ot, pds[j])
                            else:
                                nc.scalar.copy(ot, pds[j])
                            if q == 0:
                                nc.sync.dma_start(
                                    y[m * P:(m + 1) * P, dsl], ot)
                            else:
                                nc.gpsimd.dma_start(
                                    y[m * P:(m + 1) * P, dsl], ot,
                                    accum_op=ALU.add)

    nc.compile()
    return nc


def _prep_weights(w, a, b):
    """Fold LoRA into base weight (float64 accumulate, f32 round)."""
    weff = (w.astype(np.float64) + a.astype(np.float64) @ b.astype(np.float64))
    return weff.astype(np.float32)


def prep_inputs(inputs, cfg: Cfg):
    c = cfg
    x = np.asarray(inputs["x1"], np.float32).reshape(-1, c.D)
    n_tok = x.shape[0]
    assert n_tok == c.NT * c.N_CORES
    wg_e = _prep_weights(np.asarray(inputs["w_gate"], np.float32),
                         np.asarray(inputs["w_gate_lora_a"], np.float32),
                         np.asarray(inputs["w_gate_lora_b"], np.float32))
    wu_e = _prep_weights(np.asarray(inputs["w_up"], np.float32),
                         np.asarray(inputs["w_up_lora_a"], np.float32),
                         np.asarray(inputs["w_up_lora_b"], np.float32))
    wd_e = _prep_weights(np.asarray(inputs["w_down"], np.float32),
                         np.asarray(inputs["w_down_lora_a"], np.float32),
                         np.asarray(inputs["w_down_lora_b"], np.float32))
    # W[k_idx*P+kk, ft*P+ff] -> [ft, kk, k_idx, ff]
    wg_t = np.ascontiguousarray(
        wg_e.reshape(c.KC, P, c.NF, P).transpose(2, 1, 0, 3))
    wu_t = np.ascontiguousarray(
        wu_e.reshape(c.KC, P, c.NF, P).transpose(2, 1, 0, 3))
    # Wd[ft*P+ff, d*DC+dd] -> [ft, d, ff, dd]
    wd_t = np.ascontiguousarray(
        wd_e.reshape(c.NF, P, c.ND, c.DC).transpose(0, 2, 1, 3))
    bg2 = np.ascontiguousarray(
        np.asarray(inputs["b_gate"], np.float32).reshape(c.NF, P).T)
    bu2 = np.ascontiguousarray(
        np.asarray(inputs["b_up"], np.float32).reshape(c.NF, P).T)
    cst = np.zeros((2, c.D), np.float32)
    cst[0, :] = 1.0
    cst[1, :] = np.asarray(inputs["b_down"], np.float32)
    in_maps = []
    for i in range(c.N_CORES):
        xs = x[i * c.NT:(i + 1) * c.NT]
        # [NT, D] -> [kk, k_idx, m]
        xt = np.ascontiguousarray(
            xs.T.reshape(c.KC, P, c.NT).transpose(1, 0, 2))
        in_maps.append({
            "xt": xt,
            "wg": wg_t, "wu": wu_t, "wd": wd_t,
            "bg": bg2, "bu": bu2, "cst": cst,
        })
    return in_maps


_CACHE = {}


def run(inputs, trace=False, trace_kwargs=None):
    cfg = Cfg()
    b, s, d = np.asarray(inputs["x1"]).shape
    in_maps = prep_inputs(inputs, cfg)
    key = "full"
    if key not in _CACHE:
        _CACHE[key] = build_bass(cfg)
    nc = _CACHE[key]
    res = run_bass_kernel_spmd(
        nc, in_maps, list(range(cfg.N_CORES)),
        trace=trace, **(trace_kwargs or {}))
    y = np.concatenate([res.results[i]["y"] for i in range(cfg.N_CORES)], axis=0)
    return y.reshape(b, s, d).astype(np.float32), res


def kernel(**inputs) -> np.ndarray:
    out, _ = run(inputs, trace=False)
    return out


# revision 12
# speedup vs baseline: 1.0237x; 1.0237x over previous
"""Trainium2 Bass kernel for a LoRA-augmented relu-gated MLP.

Math (per reference):
    y1 = x @ w_gate + b_gate + (x @ Ag) @ Bg
    y2 = x @ w_up   + b_up   + (x @ Au) @ Bu
    x3 = relu(y1) * y2
    y3 = x3 @ w_down + b_down + (x3 @ Ad) @ Bd

Strategy:
  * Host folds every LoRA pair into its base matrix (W_eff = W + A@B in
    float64, rounded to f32) so the device kernel is a plain gated MLP.
  * Data parallel over the 8 NeuronCores: 8192 tokens -> 1024 per core,
    every core holds the full (folded) weights.
  * All matmuls run as float32r (full-rate fp32 on the PE when the moving
    dim is >= 256).
  * Per core: x is transposed on-chip (PE transpose) to xT[k, m]; the MLP
    is computed in f-quarters: gate/up produce x3T stripes [128f, NT] in
    SBUF; the down projection consumes them as stationary operands and
    accumulates partial y3 straight into DRAM via accumulate-DMA.
    b_down enters through a rank-1 ones-matmul in the first quarter.
"""

import sys
import types

import numpy as np

# The trimmed container's `antenv` lacks `axon_hooks`; bass_utils imports it
# unconditionally when tracing is requested (e.g. BASS_TRACE=1). Provide the
# degraded no-hook module so tracing falls back gracefully instead of crashing.
try:
    import antenv.axon_hooks  # noqa: F401
except ImportError:
    _m = types.ModuleType("antenv.axon_hooks")
    _m._hook = None
    _m.set_axon_ntff_profile_hook = lambda h: setattr(_m, "_hook", h)
    _m.get_axon_ntff_profile_hook = lambda: _m._hook
    sys.modules["antenv.axon_hooks"] = _m

import concourse.bacc as bacc
import concourse.bass as bass
import concourse.mybir as mybir
import concourse.tile as tile
from concourse.bass_utils import run_bass_kernel_spmd
P = 128
F32 = mybir.dt.float32
F32R = mybir.dt.float32r
AF = mybir.ActivationFunctionType
ALU = mybir.AluOpType


class Cfg:
    def __init__(self, nt=1024, d=2048, f=8192, fq=4, n_cores=8):
        assert nt % P == 0 and d % P == 0 and f % P == 0
        self.NT = nt          # tokens per core
        self.D = d            # model dim
        self.F = f            # ffn dim
        self.KC = d // P      # contraction chunks for gate/up
        self.NF = f // P      # f-tiles
        self.FQ = fq          # f quarters (x3T resident per quarter)
        assert self.NF % fq == 0
        self.SQ = self.NF // fq
        self.MH = min(512, nt)          # moving-dim chunk for gate/up
        self.NMH = nt // self.MH
        self.DC = min(512, d)           # down-proj d chunk
        self.ND = d // self.DC
        self.NM = nt // P               # token chunks of 128
        self.MG = min(8, self.NM)       # psum group size for down-proj
        self.NMG = self.NM // self.MG
        self.N_CORES = n_cores


def build_bass(cfg: Cfg):
    """Builds the per-core Bass program (same program on all cores)."""
    c = cfg
    nc = bacc.Bacc("TRN2", target_bir_lowering=False, debug=False,
                   num_swdge_queues=4)

    xt = nc.dram_tensor("xt", [P, c.KC, c.NT], F32R, kind="ExternalInput")
    wg = nc.dram_tensor("wg", [c.NF, P, c.KC, P], F32R, kind="ExternalInput")
    wu = nc.dram_tensor("wu", [c.NF, P, c.KC, P], F32R, kind="ExternalInput")
    wd = nc.dram_tensor("wd", [c.NF, c.ND, P, c.DC], F32R, kind="ExternalInput")
    bg = nc.dram_tensor("bg", [P, c.NF], F32, kind="ExternalInput")
    bu = nc.dram_tensor("bu", [P, c.NF], F32, kind="ExternalInput")
    # cst row 0: ones (first 128 entries used), row 1: b_down
    cst = nc.dram_tensor("cst", [2, c.D], F32R, kind="ExternalInput")
    y = nc.dram_tensor("y", [c.NT, c.D], F32, kind="ExternalOutput")

    with tile.TileContext(nc) as tc:
        with (
            tc.tile_pool(name="consts", bufs=1) as consts,
            tc.tile_pool(name="wpool", bufs=4) as wpool,
            tc.tile_pool(name="wdpool", bufs=6) as wdpool,
            tc.tile_pool(name="xTp", bufs=1) as xTp,
            tc.tile_pool(name="x3p", bufs=1) as x3p,
            tc.tile_pool(name="actp", bufs=2) as actp,
            tc.tile_pool(name="outp", bufs=10) as outp,
            tc.tile_pool(name="pall", bufs=1, space="PSUM") as pall,
        ):
            bgt = consts.tile([P, c.NF], F32, name="bgt")
            nc.sync.dma_start(bgt, bg[:, :])
            but = consts.tile([P, c.NF], F32, name="but")
            nc.sync.dma_start(but, bu[:, :])
            ones = consts.tile([1, P], F32R, name="ones")
            nc.sync.dma_start(ones, cst[0:1, 0:P])
            bdr = consts.tile([1, c.D], F32R, name="bdr")
            nc.sync.dma_start(bdr, cst[1:2, :])

            # ---- load pre-transposed x: xT[kk, k_idx, m] ----
            # split by token-half so the first gate/up groups can start
            # as soon as the h=0 halves land
            xT = xTp.tile([P, c.KC, c.NT], F32R, name="xT")
            for h in range(c.NMH):
                msl = slice(h * c.MH, (h + 1) * c.MH)
                for k in range(c.KC):
                    nc.sync.dma_start(xT[:, k, msl], xt[:, k, msl])

            DTAGS = ["p1", "p1", "p2", "p2", "pd0", "pd1", "pd2", "pd3"]
            for q in range(c.FQ):
                # ---- gate/up projections for this f-quarter ----
                x3 = [
                    x3p.tile([P, c.NT], F32R, tag=f"s{s}", name=f"x3_{q}_{s}")
                    for s in range(c.SQ)
                ]
                for s in range(c.SQ):
                    ft = q * c.SQ + s
                    wgt = wpool.tile([P, c.KC, P], F32R, tag="w", name=f"wg{ft}")
                    nc.sync.dma_start(wgt, wg[ft])
                    wut = wpool.tile([P, c.KC, P], F32R, tag="w", name=f"wu{ft}")
                    nc.sync.dma_start(wut, wu[ft])
                    for h in range(c.NMH):
                        msl = slice(h * c.MH, (h + 1) * c.MH)
                        p1 = pall.tile([P, c.MH], F32, tag="p1", bufs=2,
                                       name=f"p1_{ft}_{h}")
                        p2 = pall.tile([P, c.MH], F32, tag="p2", bufs=2,
                                       name=f"p2_{ft}_{h}")
                        for k in range(c.KC):
                            nc.tensor.matmul(
                                p1, wgt[:, k, :],
                                xT[:, k, msl],
                                start=(k == 0), stop=(k == c.KC - 1))
                        for k in range(c.KC):
                            nc.tensor.matmul(
                                p2, wut[:, k, :],
                                xT[:, k, msl],
                                start=(k == 0), stop=(k == c.KC - 1))
                        t1 = actp.tile([P, c.MH], F32, tag="t1", name=f"t1_{ft}_{h}")
                        nc.scalar.activation(t1, p1, AF.Relu, bias=bgt[:, ft:ft + 1])
                        # x3 = (p2 + b_up) * relu(p1 + b_gate)
                        nc.vector.scalar_tensor_tensor(
                            x3[s][:, msl], p2, but[:, ft:ft + 1], t1,
                            op0=ALU.add, op1=ALU.mult)
                # ---- down projection partials for this f-quarter ----
                for d in range(c.ND):
                    dsl = slice(d * c.DC, (d + 1) * c.DC)
                    for g in range(c.NMG):
                        pds = [
                            pall.tile([P, c.DC], F32, tag=DTAGS[j],
                                      bufs=2 if DTAGS[j] in ("p1", "p2") else 1,
                                      name=f"pd_{q}_{d}_{g}_{j}")
                            for j in range(c.MG)
                        ]
                        if q == 0:
                            # seed psum with b_down via rank-1 ones matmul
                            for j in range(c.MG):
                                nc.tensor.matmul(
                                    pds[j], ones[:, 0:P], bdr[:, dsl],
                                    start=True, stop=False)
                        for s in range(c.SQ):
                            wdt = wdpool.tile([P, c.DC], F32R, tag="wd",
                                              name=f"wd_{q}_{d}_{g}_{s}")
                            nc.sync.dma_start(wdt, wd[q * c.SQ + s, d])
                            for j in range(c.MG):
                                m = g * c.MG + j
                                nc.tensor.matmul(
                                    pds[j],
                                    x3[s][:, m * P:(m + 1) * P],
                                    wdt,
                                    start=(s == 0 and q != 0),
                                    stop=(s == c.SQ - 1))
                        for j in range(c.MG):
                            m = g * c.MG + j
                            ot = outp.tile([P, c.DC], F32, tag="ot",
                                           name=f"ot_{q}_{d}_{g}_{j}")
                            if j % 2 == 0:
                                nc.vector.tensor_copy(ot, pds[j])
                            else:
                                nc.scalar.copy(ot, pds[j])
                            if q == 0:
                                nc.sync.dma_start(
                                    y[m * P:(m + 1) * P, dsl], ot)
                            else:
                                nc.gpsimd.dma_start(
                                    y[m * P:(m + 1) * P, dsl], ot,
                                    accum_op=ALU.add)

    nc.compile()
    return nc


def _prep_weights(w, a, b):
    """Fold LoRA into base weight (float64 accumulate, f32 round)."""
    weff = (w.astype(np.float64) + a.astype(np.float64) @ b.astype(np.float64))
    return weff.astype(np.float32)


def prep_inputs(inputs, cfg: Cfg):
    c = cfg
    x = np.asarray(inputs["x1"], np.float32).reshape(-1, c.D)
    n_tok = x.shape[0]
    assert n_tok == c.NT * c.N_CORES
    wg_e = _prep_weights(np.asarray(inputs["w_gate"], np.float32),
                         np.asarray(inputs["w_gate_lora_a"], np.float32),
                         np.asarray(inputs["w_gate_lora_b"], np.float32))
    wu_e = _prep_weights(np.asarray(inputs["w_up"], np.float32),
                         np.asarray(inputs["w_up_lora_a"], np.float32),
                         np.asarray(inputs["w_up_lora_b"], np.float32))
    wd_e = _prep_weights(np.asarray(inputs["w_down"], np.float32),
                         np.asarray(inputs["w_down_lora_a"], np.float32),
                         np.asarray(inputs["w_down_lora_b"], np.float32))
    # W[k_idx*P+kk, ft*P+ff] -> [ft, kk, k_idx, ff]
    wg_t = np.ascontiguousarray(
        wg_e.reshape(c.KC, P, c.NF, P).transpose(2, 1, 0, 3))
    wu_t = np.ascontiguousarray(
        wu_e.reshape(c.KC, P, c.NF, P).transpose(2, 1, 0, 3))
    # Wd[ft*P+ff, d*DC+dd] -> [ft, d, ff, dd]
    wd_t = np.ascontiguousarray(
        wd_e.reshape(c.NF, P, c.ND, c.DC).transpose(0, 2, 1, 3))
    bg2 = np.ascontiguousarray(
        np.asarray(inputs["b_gate"], np.float32).reshape(c.NF, P).T)
    bu2 = np.ascontiguousarray(
        np.asarray(inputs["b_up"], np.float32).reshape(c.NF, P).T)
    cst = np.zeros((2, c.D), np.float32)
    cst[0, :] = 1.0
    cst[1, :] = np.asarray(inputs["b_down"], np.float32)
    in_maps = []
    for i in range(c.N_CORES):
        xs = x[i * c.NT:(i + 1) * c.NT]
        # [NT, D] -> [kk, k_idx, m]
        xt = np.ascontiguousarray(
            xs.T.reshape(c.KC, P, c.NT).transpose(1, 0, 2))
        in_maps.append({
            "xt": xt,
            "wg": wg_t, "wu": wu_t, "wd": wd_t,
            "bg": bg2, "bu": bu2, "cst": cst,
        })
    return in_maps


_CACHE = {}


def run(inputs, trace=False, trace_kwargs=None):
    cfg = Cfg()
    b, s, d = np.asarray(inputs["x1"]).shape
    in_maps = prep_inputs(inputs, cfg)
    key = "full"
    if key not in _CACHE:
        _CACHE[key] = build_bass(cfg)
    nc = _CACHE[key]
    res = run_bass_kernel_spmd(
        nc, in_maps, list(range(cfg.N_CORES)),
        trace=trace, **(trace_kwargs or {}))
    y = np.concatenate([res.results[i]["y"] for i in range(cfg.N_CORES)], axis=0)
    return y.reshape(b, s, d).astype(np.float32), res


def kernel(**inputs) -> np.ndarray:
    out, _ = run(inputs, trace=False)
    return out
